# revision 1
# baseline (speedup 1.0000x reference)
"""Trainium2 Bass kernel for a 2-layer GAT (cross-attention fusion + 8-head GAT
+ 1-head GAT) distributed over 8 NeuronCores.

Strategy (src-sharded message passing, all gathers local), v2:
  Execution on this stack is dominated by per-static-instruction dispatch
  (~40-100us each), so every phase is a tc.For_i hardware loop with a small
  fixed body; the edge schedule uses a uniform chunk count C per dst tile so
  the aggregation loop body is identical for every tile. Collectives stay
  outside dynamic loops. Dst tiles are padded to a multiple of 8 so every
  loop processes full 128-row tiles; padded rows produce den=0 partials that
  are sanitized (max(den,eps)) before the reciprocal.

  - Phase A: per-node feature transforms sharded by src node, 15x500 nodes
    per core; writes per-node gather rows [z 512 | s_src 8 | s_dst 8 | pad].
  - AllGather replicates the per-node s_dst table (256B rows).
  - Layer-1 aggregation: each core processes edges whose src it owns, C1
    chunks of 128 edge slots per global dst tile; dma_gather fetches z rows
    (local table) and s_dst rows (gathered table); one-hot matmuls
    accumulate exp-weighted sums + denominators per dst tile in PSUM.
  - ReduceScatter sums partials [30720, 520] bf16; each core ends with its
    3840-row dst range = its layer-2 src shard.
  - Normalize + ELU + z2 projection per owned node -> local layer-2 table,
    then the same aggregation scheme for layer 2 and a final ReduceScatter.
"""
import os
import sys
import math

sys.path.insert(0, "/opt/trn_rl_repo")

import numpy as np
import ml_dtypes

import concourse.bass as bass
import concourse.bacc as bacc
import concourse.tile as tile
import concourse.mybir as mybir
from concourse.bass import ts, ds
from concourse.bass_utils import run_bass_kernel_spmd

BF16 = ml_dtypes.bfloat16
NCORE = 8
F = 512          # fused dim
H = 8            # layer-1 heads
OUT = 128        # layer-2 out dim
ROW1 = 640       # layer-1 table row, bf16 elems (1280B): [z 512|s_src 8|s_dst 8|pad]
SROW = 128       # s_dst table row, bf16 elems (256B)
ROW2 = 256       # layer-2 table row (512B): [z2 128|s2src 1|pad]
PR1 = 520        # layer-1 partial row: [h 512 | den 8]
PR2 = 129        # layer-2 partial row: [h 128 | den 1]


def _ceil(a, b):
    return -(-a // b)


def _wrap_idx(idx):
    """[S] -> [128, S//16] int16, wrapped in 16 partitions, replicated 8x."""
    w = idx.reshape(-1, 16).T.astype(np.int16)
    return np.ascontiguousarray(np.tile(w, (8, 1)))


def _pack_idx(g, sd, C):
    """Per-tile packed wrapped indices: [128, T*(16C)] with [g 8C | sd 8C]."""
    wg = _wrap_idx(g)        # [128, S//16]
    wsd = _wrap_idx(sd)
    T = g.shape[0] // (C * 128)
    wg = wg.reshape(128, T, 8 * C)
    wsd = wsd.reshape(128, T, 8 * C)
    return np.ascontiguousarray(
        np.concatenate([wg, wsd], axis=2).reshape(128, T * 16 * C))


def _sched2(src, dst, shard, n_dst):
    """Uniform-chunk per-dst-tile schedule.

    Tiles padded to a multiple of 8; every tile gets exactly C chunks of 128
    edge slots on every core.  Returns (C, T, S, g[8,S], sd[8,S], dc[8,S])."""
    T = _ceil(_ceil(n_dst, 128), 8) * 8
    owner = src // shard
    percore = []
    cnt = np.zeros((NCORE, T), np.int64)
    for c in range(NCORE):
        m = owner == c
        s_loc = (src[m] - c * shard).astype(np.int64)
        d = dst[m].astype(np.int64)
        o = np.argsort(d, kind="stable")
        s_loc, d = s_loc[o], d[o]
        t = d // 128
        cnt[c] = np.bincount(t, minlength=T)
        percore.append((s_loc, d, t))
    C = max(1, int(_ceil(int(cnt.max()), 128)))
    S = T * C * 128
    g = np.zeros((NCORE, S), np.int64)
    sd = np.zeros((NCORE, S), np.int64)
    dc = np.full((NCORE, S), -1.0, np.float32)
    for c in range(NCORE):
        s_loc, d, t = percore[c]
        starts = np.searchsorted(t, np.arange(T))
        pos = np.arange(len(t)) - starts[t]
        slot = t * (C * 128) + pos
        g[c, slot] = s_loc
        sd[c, slot] = d
        dc[c, slot] = (d - t * 128).astype(np.float32)
    return C, T, S, g, sd, dc


def _prep(inputs):
    img = np.asarray(inputs["image_features"], np.float32)
    blk = np.asarray(inputs["block_features"], np.float32)
    W_img = np.asarray(inputs["W_img"], np.float32)
    W_blk = np.asarray(inputs["W_blk"], np.float32)
    Wv = np.asarray(inputs["Wv"], np.float32)
    bv = np.asarray(inputs["bv"], np.float32)
    We = np.asarray(inputs["We"], np.float32)
    be = np.asarray(inputs["be"], np.float32)
    fc1 = np.asarray(inputs["fc1"], np.float32)
    attn1 = np.asarray(inputs["attn1"], np.float32)
    fc2 = np.asarray(inputs["fc2"], np.float32)
    attn2 = np.asarray(inputs["attn2"], np.float32)
    e0s = np.asarray(inputs["edge0_src"], np.int64)
    e0d = np.asarray(inputs["edge0_dst"], np.int64)
    e1s = np.asarray(inputs["edge1_src"], np.int64)
    e1d = np.asarray(inputs["edge1_dst"], np.int64)
    ND0 = int(inputs["n_dst0"])
    ND1 = int(inputs["n_dst1"])

    NS, IMG = img.shape
    BLK = blk.shape[1]
    assert W_img.shape == (F, IMG) and W_blk.shape == (F, BLK)
    assert fc1.shape == (H, F // H, F) and fc2.shape[1] == OUT
    assert NS % NCORE == 0
    assert IMG % 128 == 0 and BLK % 128 == 0
    SS = NS // NCORE

    O = F // H
    wimgT = np.ascontiguousarray(W_img.T).astype(BF16)          # [IMG, F]
    wblkT = np.ascontiguousarray(W_blk.T).astype(BF16)          # [BLK, F]
    wv = Wv.astype(BF16)                                        # [F, F] lhsT
    we = We.astype(BF16)
    fc1T = np.ascontiguousarray(fc1.reshape(F, F).T).astype(BF16)   # [F, (h o)]
    a_src = np.einsum("hof,ho->fh", fc1, attn1[:, :O])
    a_dst = np.einsum("hof,ho->fh", fc1, attn1[:, O:])
    acat = np.concatenate([a_src, a_dst], axis=1).astype(BF16)  # [F, 16]
    fc2T = np.ascontiguousarray(fc2[0].T).astype(BF16)          # [F, OUT]
    a2s = np.tile(attn2[0, :OUT].astype(np.float32), (128, 1))  # [128, OUT]
    a2d = np.tile(attn2[0, OUT:].astype(np.float32), (128, 1))
    MB = F // 128
    biasv = np.ascontiguousarray(bv.reshape(MB, 128).T).astype(np.float32)
    biase = np.ascontiguousarray(be.reshape(MB, 128).T).astype(np.float32)
    iota = np.tile(np.arange(128, dtype=np.float32), (128, 1))
    ident = np.eye(128, dtype=np.float32).astype(BF16)

    C1, T1, S1, g1, sd1, dc1 = _sched2(e0s, e0d, SS, ND0)
    DS0p = T1 * 128 // NCORE
    C2, T2, S2, g2, sd2, dc2 = _sched2(e1s, e1d, DS0p, ND1)

    shared = dict(wimgT=wimgT, wblkT=wblkT, wv=wv, we=we, fc1T=fc1T, acat=acat,
                  fc2T=fc2T, a2s=a2s, a2d=a2d, biasv=biasv, biase=biase,
                  iota=iota, ident=ident,
                  tick=np.zeros((128, 1), np.float32))
    in_maps = []
    for c in range(NCORE):
        m = dict(shared)
        m["imgT"] = np.ascontiguousarray(
            img[c * SS:(c + 1) * SS].T).astype(BF16)
        m["blkT"] = np.ascontiguousarray(
            blk[c * SS:(c + 1) * SS].T).astype(BF16)
        m["gsd1"] = _pack_idx(g1[c], sd1[c], C1)
        m["dc1"] = np.ascontiguousarray(dc1[c].reshape(-1, 128).T)
        m["gsd2"] = _pack_idx(g2[c], sd2[c], C2)
        m["dc2"] = np.ascontiguousarray(dc2[c].reshape(-1, 128).T)
        in_maps.append(m)

    cfg = dict(NS=NS, IMG=IMG, BLK=BLK, ND0=ND0, ND1=ND1,
               SS=SS, C1=C1, T1=T1, S1=S1, C2=C2, T2=T2, S2=S2)
    return cfg, in_maps


# ---------------------------------------------------------------- device code

STOP_STAGE = int(os.environ.get("GAT_STOP_STAGE", "9"))


def _agg_layer(nc, tc, ctx, *, table, sdt, gsdp, dc1p, iota_sb,
               C, T, row, srow, prow, zw, nh, partials, rep=0):
    """Edge-aggregation For_i loop shared by both GAT layers.

    2x-unrolled body: two dst tiles per iteration with independent tile
    tags so their DMA/gather/compute chains overlap."""
    bf16 = mybir.dt.bfloat16
    f32 = mybir.dt.float32
    i16 = mybir.dt.int16
    Exp = mybir.ActivationFunctionType.Exp
    gp = ctx.enter_context(tc.tile_pool(name=f"agi{zw}_{rep}", bufs=1))
    gb = ctx.enter_context(tc.tile_pool(name=f"agb{zw}_{rep}", bufs=1))
    ph = ctx.enter_context(tc.tile_pool(name=f"agp{zw}_{rep}", bufs=1,
                                        space="PSUM"))
    nidx = C * 128
    zcol = zw + nh
    assert T % 2 == 0
    with tc.For_i(0, T // 2) as th:
      for u in range(2):
        t = th * 2 + u
        gi2 = gp.tile([128, C * 16], i16, tag=f"gi{u}")
        nc.sync.dma_start(gi2[:], gsdp[:, ts(t, C * 16)])
        dct = gp.tile([128, C], f32, tag=f"dct{u}")
        nc.sync.dma_start(dct[:], dc1p[:, ts(t, C)])
        gt = gb.tile([128, C * row], bf16, tag=f"gt{u}")
        nc.gpsimd.dma_gather(
            gt[:].rearrange("p (c e) -> p c e", e=row),
            table[:, :], gi2[:, 0:C * 8], nidx, nidx, row)
        sdt_t = gb.tile([128, C * srow], bf16, tag=f"sdt{u}")
        nc.gpsimd.dma_gather(
            sdt_t[:].rearrange("p (c e) -> p c e", e=srow),
            sdt[:, :], gi2[:, C * 8:C * 16], nidx, nidx, srow)
        g3 = gt[:].rearrange("p (c e) -> p c e", e=row)
        s3 = sdt_t[:].rearrange("p (c e) -> p c e", e=srow)
        oh = gb.tile([128, C * 128], bf16, tag=f"oh{u}")
        nc.vector.tensor_tensor(
            oh[:].rearrange("p (c d) -> p c d", d=128),
            iota_sb[:].unsqueeze(1).broadcast_to([128, C, 128]),
            dct[:].unsqueeze(2).broadcast_to([128, C, 128]),
            mybir.AluOpType.is_equal)
        esc = gb.tile([128, C * nh], f32, tag=f"esc{u}")
        e3 = esc[:].rearrange("p (c h) -> p c h", h=nh)
        nc.vector.tensor_tensor(e3, g3[:, :, zw:zw + nh], s3[:, :, 0:nh],
                                mybir.AluOpType.add)
        nc.vector.scalar_tensor_tensor(esc[:], esc[:], 0.01, esc[:],
                                       mybir.AluOpType.mult,
                                       mybir.AluOpType.max)
        zs = gb.tile([128, C * zcol], bf16, tag=f"zs{u}")
        z3 = zs[:].rearrange("p (c e) -> p c e", e=zcol)
        nc.scalar.activation(z3[:, :, zw:zcol], e3, Exp)
        nc.vector.tensor_tensor(
            z3[:, :, 0:zw].rearrange("p c (h o) -> p c h o", h=nh),
            g3[:, :, 0:zw].rearrange("p c (h o) -> p c h o", h=nh),
            z3[:, :, zw:zcol].unsqueeze(3).broadcast_to(
                [128, C, nh, zw // nh]),
            mybir.AluOpType.mult)
        st = gb.tile([128, prow], bf16, tag=f"st{u}")
        if zcol <= 512:
            # single accumulation group: rhs [z | exp]
            cur = ph.tile([128, zcol], f32, tag=f"ph{u}")
            for q in range(C):
                nc.tensor.matmul(cur[:], oh[:, q * 128:(q + 1) * 128],
                                 zs[:, q * zcol:(q + 1) * zcol],
                                 start=(q == 0), stop=(q == C - 1))
            nc.vector.tensor_copy(st[:, 0:zcol], cur[:])
        else:
            # two contiguous accumulation passes (h then den)
            cur_h = ph.tile([128, zw], f32, tag=f"phh{u}")
            cur_d = ph.tile([128, nh], f32, tag=f"phd{u}")
            for q in range(C):
                nc.tensor.matmul(cur_h[:], oh[:, q * 128:(q + 1) * 128],
                                 zs[:, q * zcol:q * zcol + zw],
                                 start=(q == 0), stop=(q == C - 1))
            for q in range(C):
                nc.tensor.matmul(cur_d[:], oh[:, q * 128:(q + 1) * 128],
                                 zs[:, q * zcol + zw:(q + 1) * zcol],
                                 start=(q == 0), stop=(q == C - 1))
            nc.vector.tensor_copy(st[:, 0:zw], cur_h[:])
            nc.vector.tensor_copy(st[:, zw:zw + nh], cur_d[:])
        nc.sync.dma_start(partials[ts(t, 128)], st[:, 0:prow])


def _build(cfg):
    stop = STOP_STAGE
    REPEAT = cfg.get("repeat", 1)
    bf16 = mybir.dt.bfloat16
    f32 = mybir.dt.float32
    i16 = mybir.dt.int16
    NS, IMG, BLK = cfg["NS"], cfg["IMG"], cfg["BLK"]
    SS = cfg["SS"]
    C1, T1, S1 = cfg["C1"], cfg["T1"], cfg["S1"]
    C2, T2, S2 = cfg["C2"], cfg["T2"], cfg["S2"]
    ND0p, ND1p = T1 * 128, T2 * 128
    DS0p, DS1p = ND0p // NCORE, ND1p // NCORE
    KI, KB, MB = IMG // 128, BLK // 128, F // 128

    nc = bacc.Bacc("TRN2", target_bir_lowering=False, debug=False,
                   enable_asserts=True, num_devices=NCORE)

    def param(name, shape, dt):
        return nc.declare_dram_parameter(name, list(shape), dt, isOutput=False)

    imgT = param("imgT", [IMG, SS], bf16)
    blkT = param("blkT", [BLK, SS], bf16)
    wimgT = param("wimgT", [IMG, F], bf16)
    wblkT = param("wblkT", [BLK, F], bf16)
    wv = param("wv", [F, F], bf16)
    we = param("we", [F, F], bf16)
    fc1T = param("fc1T", [F, F], bf16)
    acat = param("acat", [F, 16], bf16)
    fc2T = param("fc2T", [F, OUT], bf16)
    a2s = param("a2s", [128, OUT], f32)
    a2d = param("a2d", [128, OUT], f32)
    biasv = param("biasv", [128, MB], f32)
    biase = param("biase", [128, MB], f32)
    iota = param("iota", [128, 128], f32)
    ident = param("ident", [128, 128], bf16)
    gsd1 = param("gsd1", [128, S1 // 8], i16)
    dc1 = param("dc1", [128, S1 // 128], f32)
    gsd2 = param("gsd2", [128, S2 // 8], i16)
    dc2 = param("dc2", [128, S2 // 128], f32)
    tick = param("tick", [128, 1], f32)
    out = nc.declare_dram_parameter("out", [DS1p, OUT], f32, isOutput=True)
    tock = nc.declare_dram_parameter("tock", [128, 1], f32, isOutput=True)

    table1 = nc.dram_tensor("table1", [SS, ROW1], bf16)
    sdsh1 = nc.dram_tensor("sdsh1", [SS, SROW], bf16)
    sdt1 = nc.dram_tensor("sdt1", [NS, SROW], bf16, addr_space="Shared")
    sdt1b = nc.dram_tensor("sdt1b", [NS, SROW], bf16)
    partials1 = nc.dram_tensor("partials1", [ND0p, PR1], bf16)
    rs1 = nc.dram_tensor("rs1", [DS0p, PR1], bf16)
    table2 = nc.dram_tensor("table2", [DS0p, ROW2], bf16)
    sdsh2 = nc.dram_tensor("sdsh2", [DS0p, SROW], bf16)
    sdt2 = nc.dram_tensor("sdt2", [ND0p, SROW], bf16, addr_space="Shared")
    sdt2b = nc.dram_tensor("sdt2b", [ND0p, SROW], bf16)
    partials2 = nc.dram_tensor("partials2", [ND1p, PR2], bf16)
    rs2 = nc.dram_tensor("rs2", [DS1p, PR2], bf16)

    Sig = mybir.ActivationFunctionType.Sigmoid
    Exp = mybir.ActivationFunctionType.Exp
    TT = nc.vector.tensor_tensor
    MUL = mybir.AluOpType.mult
    ADD = mybir.AluOpType.add

    from contextlib import ExitStack
    with tile.TileContext(nc) as tc, ExitStack() as top:
        res = top.enter_context(tc.tile_pool(name="res", bufs=1))
        wimg_sb = res.tile([128, KI * F], bf16)
        nc.sync.dma_start(wimg_sb[:].rearrange("p (k m) -> p k m", k=KI),
                          wimgT[:, :].rearrange("(k p) m -> p k m", p=128))
        wblk_sb = res.tile([128, KB * F], bf16)
        nc.sync.dma_start(wblk_sb[:].rearrange("p (k m) -> p k m", k=KB),
                          wblkT[:, :].rearrange("(k p) m -> p k m", p=128))
        wv_sb = res.tile([128, MB * F], bf16)
        nc.sync.dma_start(wv_sb[:].rearrange("p (k m) -> p k m", k=MB),
                          wv[:, :].rearrange("(k p) m -> p k m", p=128))
        we_sb = res.tile([128, MB * F], bf16)
        nc.sync.dma_start(we_sb[:].rearrange("p (k m) -> p k m", k=MB),
                          we[:, :].rearrange("(k p) m -> p k m", p=128))
        fc1_sb = res.tile([128, MB * F], bf16)
        nc.sync.dma_start(fc1_sb[:].rearrange("p (k m) -> p k m", k=MB),
                          fc1T[:, :].rearrange("(k p) m -> p k m", p=128))
        acat_sb = res.tile([128, MB * 16], bf16)
        nc.sync.dma_start(acat_sb[:].rearrange("p (k m) -> p k m", k=MB),
                          acat[:, :].rearrange("(k p) m -> p k m", p=128))
        fc2_sb = res.tile([128, MB * OUT], bf16)
        nc.sync.dma_start(fc2_sb[:].rearrange("p (k m) -> p k m", k=MB),
                          fc2T[:, :].rearrange("(k p) m -> p k m", p=128))
        a2s_sb = res.tile([128, OUT], f32)
        nc.sync.dma_start(a2s_sb[:], a2s[:, :])
        a2d_sb = res.tile([128, OUT], f32)
        nc.sync.dma_start(a2d_sb[:], a2d[:, :])
        bv_sb = res.tile([128, MB], f32)
        nc.sync.dma_start(bv_sb[:], biasv[:, :])
        be_sb = res.tile([128, MB], f32)
        nc.sync.dma_start(be_sb[:], biase[:, :])
        iota_sb = res.tile([128, 128], f32)
        nc.sync.dma_start(iota_sb[:], iota[:, :])
        id_sb = res.tile([128, 128], bf16)
        nc.sync.dma_start(id_sb[:], ident[:, :])

        tk = res.tile([128, 1], f32)
        nc.sync.dma_start(tk[:], tick[:, :])
        nc.sync.dma_start(tock[:, :], tk[:])
        if stop < 9:
            zo = res.tile([128, OUT], f32)
            nc.vector.memset(zo[:], 0.0)
            for tt in range(DS1p // 128):
                nc.sync.dma_start(out[tt * 128:(tt + 1) * 128, :], zo[:])

        for _rep in range(REPEAT):
          # ---------------- Phase A ----------------
          WA = 500
          NT = SS // WA
          WB = 125  # block within tile (4 blocks)
          with ExitStack() as pa:
            rhsp = pa.enter_context(tc.tile_pool(name=f"parhs{_rep}", bufs=1))
            sbp = pa.enter_context(tc.tile_pool(name=f"pasb{_rep}", bufs=1))
            psp = pa.enter_context(tc.tile_pool(name=f"paps{_rep}", bufs=2,
                                                space="PSUM"))
            pst = pa.enter_context(tc.tile_pool(name=f"patr{_rep}", bufs=2,
                                                space="PSUM"))
            stp = pa.enter_context(tc.tile_pool(name=f"past{_rep}", bufs=1))
            with tc.For_i(0, NT) as nt:
              w = WA
              x_sb = rhsp.tile([128, KI * w], bf16, tag="x")
              nc.sync.dma_start(
                  x_sb[:].rearrange("p (k n) -> p k n", k=KI),
                  imgT[:, ts(nt, WA)].rearrange("(k p) n -> p k n", p=128))
              b_sb = rhsp.tile([128, KB * w], bf16, tag="b")
              nc.sync.dma_start(
                  b_sb[:].rearrange("p (k n) -> p k n", k=KB),
                  blkT[:, ts(nt, WA)].rearrange("(k p) n -> p k n", p=128))

              def mm(lhs_sb, rhs_sb, K, m, width):
                  ps = psp.tile([128, width], f32, tag="ps")
                  for k in range(K):
                      nc.tensor.matmul(
                          ps[:],
                          lhs_sb[:, (k * F + m * 128):(k * F + m * 128) + 128],
                          rhs_sb[:, k * width:(k + 1) * width],
                          start=(k == 0), stop=(k == K - 1))
                  return ps

              fi_sb = sbp.tile([128, MB * w], bf16, tag="fi")
              ti_sb = sbp.tile([128, MB * w], bf16, tag="ti")
              av_sb = sbp.tile([128, MB * w], bf16, tag="av")
              ae_sb = sbp.tile([128, MB * w], bf16, tag="ae")
              for m in range(MB):
                  ps = mm(wimg_sb, x_sb, KI, m, w)
                  nc.vector.tensor_copy(fi_sb[:, m * w:(m + 1) * w], ps[:])
              for m in range(MB):
                  ps = mm(wblk_sb, b_sb, KB, m, w)
                  nc.vector.tensor_copy(ti_sb[:, m * w:(m + 1) * w], ps[:])
              for m in range(MB):
                  ps = mm(wv_sb, fi_sb, MB, m, w)
                  nc.scalar.activation(av_sb[:, m * w:(m + 1) * w], ps[:],
                                       Sig, bias=bv_sb[:, m:m + 1])
              for m in range(MB):
                  ps = mm(we_sb, ti_sb, MB, m, w)
                  nc.scalar.activation(ae_sb[:, m * w:(m + 1) * w], ps[:],
                                       Sig, bias=be_sb[:, m:m + 1])
              fu_sb = sbp.tile([128, MB * w], bf16, tag="fu")
              TT(fu_sb[:], av_sb[:], fi_sb[:], MUL)
              TT(ae_sb[:], ae_sb[:], ti_sb[:], MUL)
              TT(fu_sb[:], fu_sb[:], ae_sb[:], ADD)
              # z and scores computed node-major: lhsT = fused (feat-part)
              for bi in range(4):
                  b0 = bi * WB
                  zps = pst.tile([128, F], f32, tag="znp")
                  for k in range(MB):
                      nc.tensor.matmul(zps[:WB, :],
                                       fu_sb[:, k * w + b0:k * w + b0 + WB],
                                       fc1_sb[:, k * F:(k + 1) * F],
                                       start=(k == 0), stop=(k == MB - 1))
                  sps = psp.tile([128, 16], f32, tag="snp")
                  for k in range(MB):
                      nc.tensor.matmul(sps[:WB, :],
                                       fu_sb[:, k * w + b0:k * w + b0 + WB],
                                       acat_sb[:, k * 16:(k + 1) * 16],
                                       start=(k == 0), stop=(k == MB - 1))
                  st = stp.tile([128, ROW1], bf16, tag="t1")
                  nc.vector.memset(st[:, F + 16:ROW1], 0.0)
                  nc.vector.tensor_copy(st[:WB, 0:F], zps[:WB, :])
                  nc.vector.tensor_copy(st[:WB, F:F + 16], sps[:WB, :])
                  nc.sync.dma_start(table1[ds(nt * WA + b0, WB)],
                                    st[:WB, :])
                  st2 = stp.tile([128, SROW], bf16, tag="t2")
                  nc.vector.memset(st2[:, 8:SROW], 0.0)
                  nc.vector.tensor_copy(st2[:WB, 0:8], st[:WB, F + 8:F + 16])
                  nc.sync.dma_start(sdsh1[ds(nt * WA + b0, WB)],
                                    st2[:WB, :])

          if stop >= 2:
              nc.gpsimd.collective_compute(
                  "AllGather", mybir.AluOpType.bypass,
                  replica_groups=[list(range(NCORE))],
                  ins=[sdsh1[:, :]], outs=[sdt1[:, :]])
              nc.sync.dma_start(sdt1b[:, :], sdt1[:, :])

          # ---------------- Layer-1 aggregation ----------------
          with ExitStack() as ag1:
            if stop >= 3:
              _agg_layer(nc, tc, ag1, table=table1, sdt=sdt1b,
                         gsdp=gsd1, dc1p=dc1, iota_sb=iota_sb,
                         C=C1, T=T1, row=ROW1, srow=SROW, prow=PR1,
                         zw=F, nh=H, partials=partials1, rep=_rep * 2)

          if stop >= 4:
              nc.gpsimd.collective_compute(
                  "ReduceScatter", ADD, replica_groups=[list(range(NCORE))],
                  ins=[partials1[:, :]], outs=[rs1[:, :]])

          # ---------------- normalize + layer-2 tables ----------------
          with ExitStack() as p4:
            if stop >= 5:
              sbp = p4.enter_context(tc.tile_pool(name=f"n2sb{_rep}", bufs=1))
              psp = p4.enter_context(tc.tile_pool(name=f"n2ps{_rep}", bufs=2,
                                                  space="PSUM"))
              ptp = p4.enter_context(tc.tile_pool(name=f"n2pt{_rep}", bufs=2,
                                                  space="PSUM"))
              with tc.For_i(0, DS0p // 128) as tt:
                  hs = sbp.tile([128, PR1], bf16, tag="hs")
                  nc.sync.dma_start(hs[:], rs1[ts(tt, 128)])
                  dent = sbp.tile([128, H], f32, tag="dent")
                  nc.vector.tensor_scalar_max(dent[:], hs[:, F:F + H], 1e-20)
                  rden = sbp.tile([128, H], f32, tag="rd")
                  nc.vector.reciprocal(rden[:], dent[:])
                  hraw = sbp.tile([128, F], f32, tag="hraw")
                  TT(hraw[:].rearrange("p (h o) -> p h o", h=H),
                     hs[:, 0:F].rearrange("p (h o) -> p h o", h=H),
                     rden[:].unsqueeze(2).broadcast_to([128, H, F // H]),
                     MUL)
                  t1 = sbp.tile([128, F], f32, tag="t1")
                  nc.vector.tensor_scalar_min(t1[:], hraw[:], 0.0)
                  nc.scalar.activation(t1[:], t1[:], Exp)
                  h1 = sbp.tile([128, F], bf16, tag="h1")
                  nc.vector.scalar_tensor_tensor(
                      h1[:], t1[:], -1.0, hraw[:],
                      ADD, mybir.AluOpType.max)
                  h1t = sbp.tile([128, MB * 128], bf16, tag="h1t")
                  for m in range(MB):
                      ptr = ptp.tile([128, 128], bf16, tag="tr")
                      nc.tensor.matmul(ptr[:, :],
                                       h1[:, m * 128:(m + 1) * 128],
                                       id_sb[:, :], is_transpose=True)
                      nc.vector.tensor_copy(h1t[:, m * 128:(m + 1) * 128],
                                            ptr[:, :])
                  pz2 = psp.tile([128, OUT], f32, tag="z2")
                  for k in range(MB):
                      nc.tensor.matmul(pz2[:, :],
                                       h1t[:, k * 128:(k + 1) * 128],
                                       fc2_sb[:, k * OUT:(k + 1) * OUT],
                                       start=(k == 0), stop=(k == MB - 1))
                  scr = sbp.tile([128, OUT], f32, tag="scr")
                  s2s = sbp.tile([128, 1], f32, tag="s2s")
                  s2d = sbp.tile([128, 1], f32, tag="s2d")
                  TT(scr[:], pz2[:], a2s_sb[:], MUL)
                  nc.vector.reduce_sum(s2s[:], scr[:], mybir.AxisListType.X)
                  TT(scr[:], pz2[:], a2d_sb[:], MUL)
                  nc.vector.reduce_sum(s2d[:], scr[:], mybir.AxisListType.X)
                  st = sbp.tile([128, ROW2], bf16, tag="st")
                  nc.vector.memset(st[:, OUT + 1:ROW2], 0.0)
                  nc.vector.tensor_copy(st[:, 0:OUT], pz2[:])
                  nc.vector.tensor_copy(st[:, OUT:OUT + 1], s2s[:])
                  nc.sync.dma_start(table2[ts(tt, 128)], st[:])
                  st2 = sbp.tile([128, SROW], bf16, tag="st2")
                  nc.vector.memset(st2[:, 1:SROW], 0.0)
                  nc.vector.tensor_copy(st2[:, 0:1], s2d[:])
                  nc.sync.dma_start(sdsh2[ts(tt, 128)], st2[:])

          if stop >= 6:
              nc.gpsimd.collective_compute(
                  "AllGather", mybir.AluOpType.bypass,
                  replica_groups=[list(range(NCORE))],
                  ins=[sdsh2[:, :]], outs=[sdt2[:, :]])
              nc.sync.dma_start(sdt2b[:, :], sdt2[:, :])

          # ---------------- Layer-2 aggregation ----------------
          with ExitStack() as ag2:
            if stop >= 7:
              _agg_layer(nc, tc, ag2, table=table2, sdt=sdt2b,
                         gsdp=gsd2, dc1p=dc2, iota_sb=iota_sb,
                         C=C2, T=T2, row=ROW2, srow=SROW, prow=PR2,
                         zw=OUT, nh=1, partials=partials2, rep=_rep * 2 + 1)

          if stop >= 8:
              nc.gpsimd.collective_compute(
                  "ReduceScatter", ADD, replica_groups=[list(range(NCORE))],
                  ins=[partials2[:, :]], outs=[rs2[:, :]])

          # ---------------- final normalize ----------------
          with ExitStack() as p8:
            if stop >= 9:
              sbp = p8.enter_context(tc.tile_pool(name=f"fsb{_rep}", bufs=1))
              with tc.For_i(0, DS1p // 128) as tt:
                  hs = sbp.tile([128, PR2], bf16, tag="hs")
                  nc.sync.dma_start(hs[:], rs2[ts(tt, 128)])
                  dent = sbp.tile([128, 1], f32, tag="dent")
                  nc.vector.tensor_scalar_max(dent[:], hs[:, OUT:OUT + 1],
                                              1e-20)
                  rden = sbp.tile([128, 1], f32, tag="rd")
                  nc.vector.reciprocal(rden[:], dent[:])
                  ot = sbp.tile([128, OUT], f32, tag="ot")
                  TT(ot[:], hs[:, 0:OUT],
                     rden[:].broadcast_to([128, OUT]), MUL)
                  nc.sync.dma_start(out[ts(tt, 128)], ot[:])

    nc.compile()
    return nc


_CACHE = {}


def _get_nc(cfg):
    key = repr(sorted((k, v) for k, v in cfg.items()))
    if key not in _CACHE:
        _CACHE[key] = _build(cfg)
    return _CACHE[key]


def kernel(**inputs) -> np.ndarray:
    cfg, in_maps = _prep(inputs)
    nc = _get_nc(cfg)
    res = run_bass_kernel_spmd(nc, in_maps, core_ids=list(range(NCORE)))
    ND1 = cfg["ND1"]
    full = np.concatenate([res.results[c]["out"] for c in range(NCORE)],
                          axis=0)
    return np.ascontiguousarray(full[:ND1])



# revision 4
# speedup vs baseline: 191.1709x; 191.1709x over previous
"""Trainium2 Bass kernel for a 2-layer GAT, v3: dst-sharded aggregation.

Design (informed by direct device timing of v2 + gather microbenchmarks):
  - dma_gather: ~4us fixed + ~12.5ns/idx at 1280B rows (~100GB/s); a single
    gather must fit its SWDGE queue's descriptor ring (~2048 descs = ~1024
    idx with 2 queues) or the device deadlocks -> all gathers <= 896 idx,
    spread over 2 queues.
  - Collectives run at ~90-140GB/s; ReduceScatters and small fixed-cost
    phases of v2 are eliminated by sharding the aggregation by DST node:
    each core owns a contiguous dst-tile range, gathers z rows per edge from
    the AllGathered z table (Shared DRAM read works), and keeps the
    aggregation local.
  - Table rows carry ONLY z (1024B L1 / 256B L2). Attention scores are
    recomputed per gathered slot on the DVE (mult + reduce against the
    replicated attention vectors) - 20-50% less gather/collective traffic.
  - Per-edge s_dst comes from chunk 0 of each tile's gather (the tile's own
    dst rows) via a transposed-one-hot PE matmul.
  - Normalize + ELU + z2 projection fused after each tile's PSUM
    accumulation; layer 2 repeats the scheme and writes normalized output.
"""
import os
import sys

sys.path.insert(0, "/opt/trn_rl_repo")

import numpy as np
import ml_dtypes

import concourse.bass as bass
import concourse.bacc as bacc
import concourse.tile as tile
import concourse.mybir as mybir
from concourse.bass import ts, ds
from concourse.bass_utils import run_bass_kernel_spmd

BF16 = ml_dtypes.bfloat16
NCORE = 8
F = 512          # fused dim
H = 8            # layer-1 heads
OUT = 128        # layer-2 out dim
ROW1 = 512       # z-table row, bf16 (1024B)
ROW2 = 128       # l2-table row, bf16 (256B)
SPLIT = 32768    # lo/hi src split (int16 gather index headroom)
GMAX = 896       # max indices per dma_gather (desc-ring safety)


def _ceil(a, b):
    return -(-a // b)


def _wrap(idx):
    """[16k] -> [128, k] int16: wrapped in 16 partitions, replicated 8x."""
    w = idx.reshape(-1, 16).T.astype(np.int16)
    return np.ascontiguousarray(np.tile(w, (8, 1)))


def _splits(nchunks):
    """Split nchunks into pieces of <= GMAX//128 chunks each."""
    lim = GMAX // 128
    out = []
    left = nchunks
    while left > 0:
        take = min(lim, left)
        out.append(take)
        left -= take
    return out


def _sched_dst(src, dst, n_dst, split, merge):
    """Dst-sharded uniform-chunk schedule (see kernel docstring).

    Per tile chunk layout: [sd (tile's own dst rows) | CL lo | CH hi].
    Returns (T, TL, CL, CH, gsd[NCORE], dc[NCORE]); gsd is [128, TL*(1+C)*8]
    int16, tile-major; index block per tile: [sd 8 | lo CL*8 | hi CH*8]
    (hi indices pre-shifted by -split).
    """
    T = _ceil(_ceil(n_dst, 128), 8) * 8
    TL = T // NCORE
    assert TL % merge == 0
    t_all = dst // 128
    split_eff = np.iinfo(np.int64).max if split is None else split
    lo_m = src < split_eff
    cl = np.bincount(t_all[lo_m], minlength=T)
    ch = np.bincount(t_all[~lo_m], minlength=T)
    CL = max(1, int(_ceil(int(cl.max()), 128)))
    CH = int(_ceil(int(ch.max()), 128)) if (~lo_m).any() else 0
    C = CL + CH

    order = np.argsort(t_all, kind="stable")
    t_s = t_all[order]
    starts = np.searchsorted(t_s, np.arange(T + 1))

    gsd_all, dc_all, dcrep_all = [], [], []
    for c in range(NCORE):
        cols = []
        dcc = np.full((TL, C, 128), -1.0, np.float32)
        for tl in range(TL):
            gt = c * TL + tl
            sd_idx = np.arange(gt * 128, (gt + 1) * 128, dtype=np.int64)
            assert split is None or gt * 128 < split
            e = order[starts[gt]:starts[gt + 1]]
            s_e = src[e]
            l_m = s_e < split_eff
            s_lo, s_hi = s_e[l_m], s_e[~l_m] - (split or 0)
            d_lo = dst[e][l_m] - gt * 128
            d_hi = dst[e][~l_m] - gt * 128
            glo = np.zeros(CL * 128, np.int64)
            glo[:len(s_lo)] = s_lo
            ghi = np.zeros(max(CH, 0) * 128, np.int64)
            ghi[:len(s_hi)] = s_hi
            dcc[tl, :CL].reshape(-1)[:len(s_lo)] = d_lo
            if CH:
                dcc[tl, CL:].reshape(-1)[:len(s_hi)] = d_hi
            cols.append(_wrap(np.concatenate([sd_idx, glo, ghi])))
        gsd_all.append(np.ascontiguousarray(np.concatenate(cols, axis=1)))
        dc_all.append(np.ascontiguousarray(
            dcc.transpose(2, 0, 1).reshape(128, TL * C)))
        dcrep_all.append(np.ascontiguousarray(
            np.tile(dcc.reshape(1, TL * C * 128), (128, 1)).astype(BF16)))
    return T, TL, CL, CH, gsd_all, dc_all, dcrep_all


def _prep(inputs):
    img = np.asarray(inputs["image_features"], np.float32)
    blk = np.asarray(inputs["block_features"], np.float32)
    W_img = np.asarray(inputs["W_img"], np.float32)
    W_blk = np.asarray(inputs["W_blk"], np.float32)
    Wv = np.asarray(inputs["Wv"], np.float32)
    bv = np.asarray(inputs["bv"], np.float32)
    We = np.asarray(inputs["We"], np.float32)
    be = np.asarray(inputs["be"], np.float32)
    fc1 = np.asarray(inputs["fc1"], np.float32)
    attn1 = np.asarray(inputs["attn1"], np.float32)
    fc2 = np.asarray(inputs["fc2"], np.float32)
    attn2 = np.asarray(inputs["attn2"], np.float32)
    e0s = np.asarray(inputs["edge0_src"], np.int64)
    e0d = np.asarray(inputs["edge0_dst"], np.int64)
    e1s = np.asarray(inputs["edge1_src"], np.int64)
    e1d = np.asarray(inputs["edge1_dst"], np.int64)
    ND0 = int(inputs["n_dst0"])
    ND1 = int(inputs["n_dst1"])

    NS, IMG = img.shape
    BLK = blk.shape[1]
    assert W_img.shape == (F, IMG) and W_blk.shape == (F, BLK)
    assert NS % NCORE == 0 and IMG % 128 == 0 and BLK % 128 == 0
    SS = NS // NCORE
    O = F // H

    wimgT = np.ascontiguousarray(W_img.T).astype(BF16)          # [IMG, F]
    wblkT = np.ascontiguousarray(W_blk.T).astype(BF16)          # [BLK, F]
    wv = Wv.astype(BF16)
    we = We.astype(BF16)
    fc1T = np.ascontiguousarray(fc1.reshape(F, F).T).astype(BF16)
    # attention vectors replicated across partitions, [128, F] / [128, OUT]
    a1s = np.tile(attn1[:, :O].reshape(-1).astype(np.float32), (128, 1))
    a1d = np.tile(attn1[:, O:].reshape(-1).astype(np.float32), (128, 1))
    a2s = np.tile(attn2[0, :OUT].astype(np.float32), (128, 1))
    a2d = np.tile(attn2[0, OUT:].astype(np.float32), (128, 1))
    fc2T = np.ascontiguousarray(fc2[0].T).astype(BF16)          # [F, OUT]
    MB = F // 128
    biasv = np.ascontiguousarray(bv.reshape(MB, 128).T).astype(np.float32)
    biase = np.ascontiguousarray(be.reshape(MB, 128).T).astype(np.float32)
    iota = np.tile(np.arange(128, dtype=np.float32), (128, 1))
    iotaT = np.tile(np.arange(128, dtype=np.float32)[:, None],
                    (1, 128)).astype(BF16)
    ident = np.eye(128, dtype=np.float32).astype(BF16)

    MG1, MG2 = 2, 2
    T1, TL1, CL1, CH1, gsd1, dc1, dcrep1 = _sched_dst(e0s, e0d, ND0, SPLIT,
                                                       MG1)
    T2, TL2, CL2, CH2, gsd2, dc2, dcrep2 = _sched_dst(e1s, e1d, ND1, None,
                                                      MG2)
    assert CH2 == 0

    shared = dict(wimgT=wimgT, wblkT=wblkT, wv=wv, we=we, fc1T=fc1T,
                  fc2T=fc2T, a1s=a1s, a1d=a1d, a2s=a2s, a2d=a2d,
                  biasv=biasv, biase=biase, iota=iota, iotaT=iotaT,
                  ident=ident,
                  tick=np.zeros((128, 1), np.float32))
    in_maps = []
    for c in range(NCORE):
        m = dict(shared)
        m["imgT"] = np.ascontiguousarray(img[c * SS:(c + 1) * SS].T).astype(BF16)
        m["blkT"] = np.ascontiguousarray(blk[c * SS:(c + 1) * SS].T).astype(BF16)
        m["gsd1"] = gsd1[c]
        m["dc1"] = dc1[c]
        m["dcrep1"] = dcrep1[c]
        m["gsd2"] = gsd2[c]
        m["dc2"] = dc2[c]
        m["dcrep2"] = dcrep2[c]
        in_maps.append(m)

    cfg = dict(NS=NS, IMG=IMG, BLK=BLK, ND0=ND0, ND1=ND1, SS=SS,
               T1=T1, TL1=TL1, CL1=CL1, CH1=CH1, MG1=MG1,
               T2=T2, TL2=TL2, CL2=CL2, MG2=MG2)
    return cfg, in_maps


# ---------------------------------------------------------------- device code

STOP_STAGE = int(os.environ.get("GAT_STOP_STAGE", "9"))


def _build(cfg):
    stop = STOP_STAGE
    REPEAT = cfg.get("repeat", 1)
    ONLY = cfg.get("only")
    bf16 = mybir.dt.bfloat16
    f32 = mybir.dt.float32
    i16 = mybir.dt.int16
    NS, IMG, BLK, SS = cfg["NS"], cfg["IMG"], cfg["BLK"], cfg["SS"]
    T1, TL1, CL1, CH1, MG1 = (cfg["T1"], cfg["TL1"], cfg["CL1"], cfg["CH1"],
                              cfg["MG1"])
    T2, TL2, CL2, MG2 = cfg["T2"], cfg["TL2"], cfg["CL2"], cfg["MG2"]
    C1 = CL1 + CH1
    C2 = CL2
    KI, KB, MB = IMG // 128, BLK // 128, F // 128

    nc = bacc.Bacc("TRN2", target_bir_lowering=False, debug=False,
                   enable_asserts=True, num_devices=NCORE,
                   num_swdge_queues=2)

    def param(name, shape, dt):
        return nc.declare_dram_parameter(name, list(shape), dt, isOutput=False)

    imgT = param("imgT", [IMG, SS], bf16)
    blkT = param("blkT", [BLK, SS], bf16)
    wimgT = param("wimgT", [IMG, F], bf16)
    wblkT = param("wblkT", [BLK, F], bf16)
    wv = param("wv", [F, F], bf16)
    we = param("we", [F, F], bf16)
    fc1T = param("fc1T", [F, F], bf16)
    fc2T = param("fc2T", [F, OUT], bf16)
    a1s = param("a1s", [128, F], f32)
    a1d = param("a1d", [128, F], f32)
    a2s = param("a2s", [128, OUT], f32)
    a2d = param("a2d", [128, OUT], f32)
    biasv = param("biasv", [128, MB], f32)
    biase = param("biase", [128, MB], f32)
    iota = param("iota", [128, 128], f32)
    iotaT = param("iotaT", [128, 128], bf16)
    ident = param("ident", [128, 128], bf16)
    gsd1 = param("gsd1", [128, TL1 * (1 + C1) * 8], i16)
    dc1 = param("dc1", [128, TL1 * C1], f32)
    dcrep1 = param("dcrep1", [128, TL1 * C1 * 128], bf16)
    gsd2 = param("gsd2", [128, TL2 * (1 + C2) * 8], i16)
    dc2 = param("dc2", [128, TL2 * C2], f32)
    dcrep2 = param("dcrep2", [128, TL2 * C2 * 128], bf16)
    tick = param("tick", [128, 1], f32)
    out = nc.declare_dram_parameter("out", [TL2 * 128, OUT], f32, isOutput=True)
    tock = nc.declare_dram_parameter("tock", [128, 1], f32, isOutput=True)

    tzsh = nc.dram_tensor("tzsh", [SS, ROW1], bf16)
    tz_all = nc.dram_tensor("tz_all", [NS, ROW1], bf16, addr_space="Shared")
    t2sh = nc.dram_tensor("t2sh", [TL1 * 128, ROW2], bf16)
    t2all = nc.dram_tensor("t2all", [T1 * 128, ROW2], bf16, addr_space="Shared")

    Sig = mybir.ActivationFunctionType.Sigmoid
    Exp = mybir.ActivationFunctionType.Exp
    TT = nc.vector.tensor_tensor
    MUL = mybir.AluOpType.mult
    ADD = mybir.AluOpType.add
    AX = mybir.AxisListType.X

    from contextlib import ExitStack
    with tile.TileContext(nc) as tc, ExitStack() as top:
        res = top.enter_context(tc.tile_pool(name="res", bufs=1))

        def load_km(t, src, K, width):
            nc.sync.dma_start(t[:].rearrange("p (k m) -> p k m", k=K),
                              src[:, :].rearrange("(k p) m -> p k m", p=128))

        wimg_sb = res.tile([128, KI * F], bf16)
        load_km(wimg_sb, wimgT, KI, F)
        wblk_sb = res.tile([128, KB * F], bf16)
        load_km(wblk_sb, wblkT, KB, F)
        wv_sb = res.tile([128, MB * F], bf16)
        load_km(wv_sb, wv, MB, F)
        we_sb = res.tile([128, MB * F], bf16)
        load_km(we_sb, we, MB, F)
        fc1_sb = res.tile([128, MB * F], bf16)
        load_km(fc1_sb, fc1T, MB, F)
        fc2_sb = res.tile([128, MB * OUT], bf16)
        load_km(fc2_sb, fc2T, MB, OUT)
        a1s_sb = res.tile([128, F], f32)
        nc.sync.dma_start(a1s_sb[:], a1s[:, :])
        a1d_sb = res.tile([128, F], f32)
        nc.sync.dma_start(a1d_sb[:], a1d[:, :])
        a2s_sb = res.tile([128, OUT], f32)
        nc.sync.dma_start(a2s_sb[:], a2s[:, :])
        a2d_sb = res.tile([128, OUT], f32)
        nc.sync.dma_start(a2d_sb[:], a2d[:, :])
        bv_sb = res.tile([128, MB], f32)
        nc.sync.dma_start(bv_sb[:], biasv[:, :])
        be_sb = res.tile([128, MB], f32)
        nc.sync.dma_start(be_sb[:], biase[:, :])
        iota_sb = res.tile([128, 128], f32)
        nc.sync.dma_start(iota_sb[:], iota[:, :])
        iotaT_sb = res.tile([128, 128], bf16)
        nc.sync.dma_start(iotaT_sb[:], iotaT[:, :])
        id_sb = res.tile([128, 128], bf16)
        nc.sync.dma_start(id_sb[:], ident[:, :])

        tk = res.tile([128, 1], f32)
        nc.sync.dma_start(tk[:], tick[:, :])
        nc.sync.dma_start(tock[:, :], tk[:])
        if stop < 5:
            zo = res.tile([128, OUT], f32)
            nc.vector.memset(zo[:], 0.0)
            for tt in range(TL2):
                nc.sync.dma_start(out[ts(tt, 128)], zo[:])

        for _rep in range(REPEAT):
          def on(s, _rep=_rep):
              return stop >= s and (ONLY is None or _rep == 0 or ONLY == s)

          # ---------------- Phase A: per-node transforms ----------------
          WA = 500
          NT = SS // WA
          WB = 125
          if on(1):
           with ExitStack() as pa:
            rhsp = pa.enter_context(tc.tile_pool(name=f"parhs{_rep}", bufs=1))
            sbp = pa.enter_context(tc.tile_pool(name=f"pasb{_rep}", bufs=1))
            psp = pa.enter_context(tc.tile_pool(name=f"paps{_rep}", bufs=2,
                                                space="PSUM"))
            pst = pa.enter_context(tc.tile_pool(name=f"patr{_rep}", bufs=2,
                                                space="PSUM"))
            stp = pa.enter_context(tc.tile_pool(name=f"past{_rep}", bufs=1))
            with tc.For_i(0, NT) as nt:
              w = WA
              x_sb = rhsp.tile([128, KI * w], bf16, tag="x")
              nc.sync.dma_start(
                  x_sb[:].rearrange("p (k n) -> p k n", k=KI),
                  imgT[:, ts(nt, WA)].rearrange("(k p) n -> p k n", p=128))
              b_sb = rhsp.tile([128, KB * w], bf16, tag="b")
              nc.sync.dma_start(
                  b_sb[:].rearrange("p (k n) -> p k n", k=KB),
                  blkT[:, ts(nt, WA)].rearrange("(k p) n -> p k n", p=128))

              def mm(lhs_sb, rhs_sb, K, m, width):
                  ps = psp.tile([128, width], f32, tag="ps")
                  for k in range(K):
                      nc.tensor.matmul(
                          ps[:],
                          lhs_sb[:, (k * F + m * 128):(k * F + m * 128) + 128],
                          rhs_sb[:, k * width:(k + 1) * width],
                          start=(k == 0), stop=(k == K - 1))
                  return ps

              fi_sb = sbp.tile([128, MB * w], bf16, tag="fi")
              ti_sb = sbp.tile([128, MB * w], bf16, tag="ti")
              av_sb = sbp.tile([128, MB * w], bf16, tag="av")
              ae_sb = sbp.tile([128, MB * w], bf16, tag="ae")
              for m in range(MB):
                  ps = mm(wimg_sb, x_sb, KI, m, w)
                  nc.vector.tensor_copy(fi_sb[:, m * w:(m + 1) * w], ps[:])
              for m in range(MB):
                  ps = mm(wblk_sb, b_sb, KB, m, w)
                  nc.vector.tensor_copy(ti_sb[:, m * w:(m + 1) * w], ps[:])
              for m in range(MB):
                  ps = mm(wv_sb, fi_sb, MB, m, w)
                  nc.scalar.activation(av_sb[:, m * w:(m + 1) * w], ps[:],
                                       Sig, bias=bv_sb[:, m:m + 1])
              for m in range(MB):
                  ps = mm(we_sb, ti_sb, MB, m, w)
                  nc.scalar.activation(ae_sb[:, m * w:(m + 1) * w], ps[:],
                                       Sig, bias=be_sb[:, m:m + 1])
              fu_sb = sbp.tile([128, MB * w], bf16, tag="fu")
              TT(fu_sb[:], av_sb[:], fi_sb[:], MUL)
              TT(ae_sb[:], ae_sb[:], ti_sb[:], MUL)
              TT(fu_sb[:], fu_sb[:], ae_sb[:], ADD)
              for bi in range(4):
                  b0 = bi * WB
                  zps = pst.tile([128, F], f32, tag="znp")
                  for k in range(MB):
                      nc.tensor.matmul(zps[:WB, :],
                                       fu_sb[:, k * w + b0:k * w + b0 + WB],
                                       fc1_sb[:, k * F:(k + 1) * F],
                                       start=(k == 0), stop=(k == MB - 1))
                  st = stp.tile([128, ROW1], bf16, tag="t1")
                  nc.vector.tensor_copy(st[:WB, :], zps[:WB, :])
                  nc.sync.dma_start(tzsh[ds(nt * WA + b0, WB)], st[:WB, :])

          if on(2):
              nc.gpsimd.collective_compute(
                  "AllGather", mybir.AluOpType.bypass,
                  replica_groups=[list(range(NCORE))],
                  ins=[tzsh[:, :]], outs=[tz_all[:, :]])

          # ---------------- Layer-1 agg + fused norm/ELU/z2 ----------------
          if on(3):
           with ExitStack() as ag1:
            gp = ag1.enter_context(tc.tile_pool(name=f"g1i{_rep}", bufs=1))
            gb = ag1.enter_context(tc.tile_pool(name=f"g1b{_rep}", bufs=1))
            ph = ag1.enter_context(tc.tile_pool(name=f"g1p{_rep}", bufs=1,
                                                space="PSUM"))
            pt = ag1.enter_context(tc.tile_pool(name=f"g1t{_rep}", bufs=2,
                                                space="PSUM"))
            NC1 = 1 + C1             # chunks per tile incl. sd chunk
            W1 = NC1 * 8             # index cols per tile
            with tc.For_i(0, TL1 // MG1) as mp:
              gil = gp.tile([128, MG1 * W1], i16, tag="gi")
              nc.sync.dma_start(gil[:], gsd1[:, ts(mp, MG1 * W1)])
              dct = gp.tile([128, MG1 * C1], f32, tag="dct")
              nc.sync.dma_start(dct[:], dc1[:, ts(mp, MG1 * C1)])
              dcr = gp.tile([128, MG1 * C1 * 128], bf16, tag="dcr")
              nc.sync.dma_start(dcr[:], dcrep1[:, ts(mp, MG1 * C1 * 128)])
              for u in range(MG1):
                gz = gb.tile([128, NC1 * ROW1], bf16, tag=f"gz{u}")
                gz3 = gz[:].rearrange("p (c e) -> p c e", e=ROW1)
                qq = 0
                cc = 0
                for ns in _splits(NC1):
                    lim = 1 + CL1    # lo rows end after chunk CL1 (sd incl.)
                    # pick table base: chunks [cc, cc+ns) all-lo or all-hi?
                    if cc + ns <= lim:
                        base = tz_all[:, :]
                    elif cc >= lim:
                        base = tz_all[SPLIT:, :]
                    else:
                        # split crossing the lo/hi boundary
                        ns_a = lim - cc
                        for ns2, b2 in ((ns_a, tz_all[:, :]),
                                        (ns - ns_a, tz_all[SPLIT:, :])):
                            nc.gpsimd.dma_gather(
                                gz3[:, cc:cc + ns2, :], b2,
                                gil[:, u * W1 + cc * 8:u * W1 + (cc + ns2) * 8],
                                ns2 * 128, ns2 * 128, ROW1, queue_num=qq % 2)
                            qq += 1
                            cc += ns2
                        continue
                    nc.gpsimd.dma_gather(
                        gz3[:, cc:cc + ns, :], base,
                        gil[:, u * W1 + cc * 8:u * W1 + (cc + ns) * 8],
                        ns * 128, ns * 128, ROW1, queue_num=qq % 2)
                    qq += 1
                    cc += ns
                # scores: s_src for edge chunks, s_dst for chunk 0
                tmp = gb.tile([128, C1 * F], f32, tag="tmp")
                TT(tmp[:].rearrange("p (c f) -> p c f", f=F),
                   gz3[:, 1:NC1, :],
                   a1s_sb[:].unsqueeze(1).broadcast_to([128, C1, F]), MUL)
                esc = gb.tile([128, C1 * 8], f32, tag=f"esc{u}")
                nc.vector.reduce_sum(
                    esc[:].rearrange("p (c h) -> p c h", h=H).unsqueeze(3),
                    tmp[:].rearrange("p (c h o) -> p c h o", h=H, o=F // H),
                    AX)
                tsd = gb.tile([128, F], f32, tag="tsd")
                TT(tsd[:], gz3[:, 0, :], a1d_sb[:], MUL)
                sdf = gb.tile([128, H], f32, tag="sdf")
                nc.vector.reduce_sum(
                    sdf[:].rearrange("p (h x) -> p h x", x=1),
                    tsd[:].rearrange("p (h o) -> p h o", h=H), AX)
                sd = gb.tile([128, H], bf16, tag="sd")
                nc.vector.tensor_copy(sd[:], sdf[:])
                oh = gb.tile([128, C1 * 128], bf16, tag=f"oh{u}")
                TT(oh[:].rearrange("p (c d) -> p c d", d=128),
                   iota_sb[:].unsqueeze(1).broadcast_to([128, C1, 128]),
                   dct[:, u * C1:(u + 1) * C1].unsqueeze(2)
                       .broadcast_to([128, C1, 128]),
                   mybir.AluOpType.is_equal)
                ohT = gb.tile([128, C1 * 128], bf16, tag=f"ohT{u}")
                TT(ohT[:].rearrange("p (c q) -> p c q", q=128),
                   iotaT_sb[:].unsqueeze(1).broadcast_to([128, C1, 128]),
                   dcr[:, u * C1 * 128:(u + 1) * C1 * 128]
                       .rearrange("p (c q) -> p c q", q=128),
                   mybir.AluOpType.is_equal)
                sdpe = ph.tile([128, C1 * 8], f32, tag=f"sdpe{u}")
                for c in range(C1):
                    nc.tensor.matmul(sdpe[:, c * 8:(c + 1) * 8],
                                     ohT[:, c * 128:(c + 1) * 128], sd[:])
                TT(esc[:], esc[:], sdpe[:], ADD)
                nc.vector.scalar_tensor_tensor(esc[:], esc[:], 0.01, esc[:],
                                               MUL, mybir.AluOpType.max)
                zs = gb.tile([128, C1 * 520], bf16, tag=f"zs{u}")
                z3 = zs[:].rearrange("p (c e) -> p c e", e=520)
                nc.scalar.activation(
                    z3[:, :, F:F + 8],
                    esc[:].rearrange("p (c h) -> p c h", h=H), Exp)
                TT(z3[:, :, 0:F].rearrange("p c (h o) -> p c h o", h=H),
                   gz3[:, 1:NC1, :].rearrange("p c (h o) -> p c h o", h=H),
                   z3[:, :, F:F + 8].unsqueeze(3)
                       .broadcast_to([128, C1, H, F // H]), MUL)
                cur_h = ph.tile([128, F], f32, tag=f"ch{u}")
                cur_d = ph.tile([128, 8], f32, tag=f"cd{u}")
                for c in range(C1):
                    nc.tensor.matmul(cur_h[:], oh[:, c * 128:(c + 1) * 128],
                                     zs[:, c * 520:c * 520 + F],
                                     start=(c == 0), stop=(c == C1 - 1))
                for c in range(C1):
                    nc.tensor.matmul(cur_d[:], oh[:, c * 128:(c + 1) * 128],
                                     zs[:, c * 520 + F:(c + 1) * 520],
                                     start=(c == 0), stop=(c == C1 - 1))
                rden = gb.tile([128, 8], f32, tag=f"rd{u}")
                nc.vector.tensor_scalar_max(rden[:], cur_d[:], 1e-20)
                nc.vector.reciprocal(rden[:], rden[:])
                hraw = gb.tile([128, F], f32, tag=f"hraw{u}")
                TT(hraw[:].rearrange("p (h o) -> p h o", h=H),
                   cur_h[:].rearrange("p (h o) -> p h o", h=H),
                   rden[:].unsqueeze(2).broadcast_to([128, H, F // H]), MUL)
                t1 = gb.tile([128, F], f32, tag=f"t1{u}")
                nc.vector.tensor_scalar_min(t1[:], hraw[:], 0.0)
                nc.scalar.activation(t1[:], t1[:], Exp)
                h1 = gb.tile([128, F], bf16, tag=f"h1{u}")
                nc.vector.scalar_tensor_tensor(h1[:], t1[:], -1.0, hraw[:],
                                               ADD, mybir.AluOpType.max)
                h1t = gb.tile([128, MB * 128], bf16, tag=f"h1t{u}")
                for m in range(MB):
                    trm = pt.tile([128, 128], bf16, tag="tr")
                    nc.tensor.matmul(trm[:, :], h1[:, m * 128:(m + 1) * 128],
                                     id_sb[:, :], is_transpose=True)
                    nc.vector.tensor_copy(h1t[:, m * 128:(m + 1) * 128],
                                          trm[:, :])
                z2a = ph.tile([128, OUT], f32, tag=f"ch{u}")
                for k in range(MB):
                    nc.tensor.matmul(z2a[:], h1t[:, k * 128:(k + 1) * 128],
                                     fc2_sb[:, k * OUT:(k + 1) * OUT],
                                     start=(k == 0), stop=(k == MB - 1))
                st = gb.tile([128, ROW2], bf16, tag=f"st{u}")
                nc.vector.tensor_copy(st[:], z2a[:])
                nc.sync.dma_start(t2sh[ds(mp * (MG1 * 128) + u * 128, 128)],
                                  st[:])

          if on(4):
              nc.gpsimd.collective_compute(
                  "AllGather", mybir.AluOpType.bypass,
                  replica_groups=[list(range(NCORE))],
                  ins=[t2sh[:, :]], outs=[t2all[:, :]])

          # ---------------- Layer-2 agg + fused final normalize ------------
          if on(5):
           with ExitStack() as ag2:
            gp = ag2.enter_context(tc.tile_pool(name=f"g2i{_rep}", bufs=1))
            gb = ag2.enter_context(tc.tile_pool(name=f"g2b{_rep}", bufs=1))
            ph = ag2.enter_context(tc.tile_pool(name=f"g2p{_rep}", bufs=1,
                                                space="PSUM"))
            pt = ag2.enter_context(tc.tile_pool(name=f"g2t{_rep}", bufs=2,
                                                space="PSUM"))
            NC2 = 1 + C2
            W2 = NC2 * 8
            with tc.For_i(0, TL2 // MG2) as mp:
              gil = gp.tile([128, MG2 * W2], i16, tag="gi")
              nc.sync.dma_start(gil[:], gsd2[:, ts(mp, MG2 * W2)])
              dct = gp.tile([128, MG2 * C2], f32, tag="dct")
              nc.sync.dma_start(dct[:], dc2[:, ts(mp, MG2 * C2)])
              dcr = gp.tile([128, MG2 * C2 * 128], bf16, tag="dcr")
              nc.sync.dma_start(dcr[:], dcrep2[:, ts(mp, MG2 * C2 * 128)])
              for u in range(MG2):
                gz = gb.tile([128, NC2 * ROW2], bf16, tag=f"gz{u}")
                gz3 = gz[:].rearrange("p (c e) -> p c e", e=ROW2)
                qq = 0
                cc = 0
                for ns in _splits(NC2):
                    nc.gpsimd.dma_gather(
                        gz3[:, cc:cc + ns, :], t2all[:, :],
                        gil[:, u * W2 + cc * 8:u * W2 + (cc + ns) * 8],
                        ns * 128, ns * 128, ROW2, queue_num=qq % 2)
                    qq += 1
                    cc += ns
                tmp = gb.tile([128, C2 * OUT], f32, tag="tmp")
                TT(tmp[:].rearrange("p (c f) -> p c f", f=OUT),
                   gz3[:, 1:NC2, :],
                   a2s_sb[:].unsqueeze(1).broadcast_to([128, C2, OUT]), MUL)
                esc = gb.tile([128, C2], f32, tag=f"esc{u}")
                nc.vector.reduce_sum(esc[:].unsqueeze(2),
                                     tmp[:].rearrange("p (c f) -> p c f",
                                                      f=OUT), AX)
                tsd = gb.tile([128, OUT], f32, tag="tsd")
                TT(tsd[:], gz3[:, 0, :], a2d_sb[:], MUL)
                sdf = gb.tile([128, 1], f32, tag="sdf")
                nc.vector.reduce_sum(sdf[:].unsqueeze(2),
                                     tsd[:].unsqueeze(1), AX)
                sd = gb.tile([128, 1], bf16, tag="sd")
                nc.vector.tensor_copy(sd[:], sdf[:])
                oh = gb.tile([128, C2 * 128], bf16, tag=f"oh{u}")
                TT(oh[:].rearrange("p (c d) -> p c d", d=128),
                   iota_sb[:].unsqueeze(1).broadcast_to([128, C2, 128]),
                   dct[:, u * C2:(u + 1) * C2].unsqueeze(2)
                       .broadcast_to([128, C2, 128]),
                   mybir.AluOpType.is_equal)
                ohT = gb.tile([128, C2 * 128], bf16, tag=f"ohT{u}")
                TT(ohT[:].rearrange("p (c q) -> p c q", q=128),
                   iotaT_sb[:].unsqueeze(1).broadcast_to([128, C2, 128]),
                   dcr[:, u * C2 * 128:(u + 1) * C2 * 128]
                       .rearrange("p (c q) -> p c q", q=128),
                   mybir.AluOpType.is_equal)
                sdpe = ph.tile([128, C2], f32, tag=f"sdpe{u}")
                for c in range(C2):
                    nc.tensor.matmul(sdpe[:, c:c + 1],
                                     ohT[:, c * 128:(c + 1) * 128], sd[:])
                TT(esc[:], esc[:], sdpe[:], ADD)
                nc.vector.scalar_tensor_tensor(esc[:], esc[:], 0.01, esc[:],
                                               MUL, mybir.AluOpType.max)
                zs = gb.tile([128, C2 * (OUT + 1)], bf16, tag=f"zs{u}")
                z3 = zs[:].rearrange("p (c e) -> p c e", e=OUT + 1)
                nc.scalar.activation(z3[:, :, OUT:OUT + 1],
                                     esc[:].unsqueeze(2), Exp)
                TT(z3[:, :, 0:OUT], gz3[:, 1:NC2, :],
                   z3[:, :, OUT:OUT + 1].broadcast_to([128, C2, OUT]), MUL)
                cur = ph.tile([128, OUT + 1], f32, tag=f"cur{u}")
                for c in range(C2):
                    nc.tensor.matmul(cur[:], oh[:, c * 128:(c + 1) * 128],
                                     zs[:, c * (OUT + 1):(c + 1) * (OUT + 1)],
                                     start=(c == 0), stop=(c == C2 - 1))
                rden = gb.tile([128, 1], f32, tag=f"rd{u}")
                nc.vector.tensor_scalar_max(rden[:], cur[:, OUT:OUT + 1],
                                            1e-20)
                nc.vector.reciprocal(rden[:], rden[:])
                ot = gb.tile([128, OUT], f32, tag=f"ot{u}")
                TT(ot[:], cur[:, 0:OUT], rden[:].broadcast_to([128, OUT]), MUL)
                nc.sync.dma_start(out[ds(mp * (MG2 * 128) + u * 128, 128)],
                                  ot[:])

    nc.compile()
    return nc


_CACHE = {}


def _get_nc(cfg):
    key = repr(sorted((k, v) for k, v in cfg.items()))
    if key not in _CACHE:
        _CACHE[key] = _build(cfg)
    return _CACHE[key]


def kernel(**inputs) -> np.ndarray:
    cfg, in_maps = _prep(inputs)
    nc = _get_nc(cfg)
    res = run_bass_kernel_spmd(nc, in_maps, core_ids=list(range(NCORE)))
    ND1 = cfg["ND1"]
    full = np.concatenate([res.results[c]["out"] for c in range(NCORE)],
                          axis=0)
    return np.ascontiguousarray(full[:ND1])
